# revision 3
# baseline (speedup 1.0000x reference)
"""Sliding context-window gather kernel for Trainium2 (Bass/Tile).

Computes, for x[B=32, T=2000, C=80] and lengths[B]:
    out[b, t, c*11 + i] = x[b, t + i - 5, c]          (zero outside [0, T))
                          * (t < round(T * lengths[b]))
i.e. an 11-tap sliding-window gather along T with channel-major
interleave, masked by per-sample length.

Sharding: pure data-parallel, 4 samples per core across 8 cores.

Per-core layout: the host zero-pads x by 5 rows on each side of T, and
each sample is loaded into SBUF as an overlapping-window view
[125 partitions x 26 rows x 80 ch] (one contiguous-chunk DMA; partition
p holds padded rows 16p .. 16p+25, i.e. t = 16p-5 .. 16p+20).

The interleaved output row for folded row q is built with ONE fused
op per q-row:   O[p, (c,i)] = X[p, q+i, c] * mask[p, q]
as a tensor_scalar multiply with a per-partition scalar (the mask value
for that q-row) and a transposed source access pattern (c outer stride
1, i inner stride 80), writing the full 880-wide output row
contiguously.  The 16 q-rows per sample are round-robined across the
vector, scalar (activation) and gpsimd engines so the three engines
build the output in parallel; DMA stores (sync queue) stream out every
8 q-rows (3.52 MB contiguous-per-partition transfers).  All HBM traffic
is contiguous-chunk DMA: ~1.04 MB load + 7.04 MB store per sample.
"""

import numpy as np

import concourse.mybir as mybir
from concourse import bacc, bass
from concourse.ap import AP
from concourse.bass_utils import run_bass_kernel_spmd
from concourse.tile import TileContext

LEFT = 5
RIGHT = 5
CTXW = LEFT + RIGHT + 1  # 11
B, T, C = 32, 2000, 80
W = C * CTXW  # 880
N_CORES = 8
B_LOC = B // N_CORES  # 4 samples per core
P = 125  # SBUF partitions used per sample fold
Q = 16   # consecutive t rows per partition (P * Q == T)
QG = Q + LEFT + RIGHT  # 26 rows per partition incl. halo
TP = T + LEFT + RIGHT  # padded time length
F32 = mybir.dt.float32

assert P * Q == T

# q-rows per store chunk (must divide Q)
QS = 8


def _build_bass(variant: str = "full"):
    nc = bacc.Bacc()
    xp_dram = nc.declare_dram_parameter("xp", [B_LOC, TP, C], F32, isOutput=False)
    msk = nc.declare_dram_parameter("mask", [B_LOC, T], F32, isOutput=False)
    out = nc.declare_dram_parameter("out", [B_LOC, T, W], F32, isOutput=True)

    engines = [None, None, None]  # filled once nc exists

    with TileContext(nc) as tc:
        with (
            tc.tile_pool(name="xpool", bufs=1) as xpool,
            tc.tile_pool(name="mpool", bufs=1) as mpool,
            tc.tile_pool(name="opool", bufs=1) as opool,
        ):
            X = [None] * B_LOC
            M = [None] * B_LOC
            # prefetch all four samples' inputs up front on the scalar
            # (ACT) HWDGE queue; stores go on the sync (SP) queue so
            # loads never queue behind stores.
            for b in range(B_LOC):
                X[b] = xpool.tile([P, QG, C], F32, tag=f"X{b}", name=f"X{b}")
                M[b] = mpool.tile([P, Q], F32, tag=f"M{b}", name=f"M{b}")
                # overlapping window view: element (p, r, c) reads
                # x_pad[b, Q*p + r, c]  (rows overlap across partitions)
                window = AP(
                    xp_dram[b].tensor,
                    b * TP * C,
                    [[Q * C, P], [C, QG], [1, C]],
                )
                nc.scalar.dma_start(out=X[b], in_=window)
                nc.scalar.dma_start(
                    out=M[b], in_=msk[b].rearrange("(p q) -> p q", q=Q)
                )

            eng_idx = 0
            for b in range(B_LOC):
                out_b = out[b].rearrange("(p q) w -> p q w", q=Q)
                for h in range(Q // QS):
                    j0 = h * QS
                    O = opool.tile([P, QS, W], F32, tag=f"O{(b * (Q // QS) + h) % 4}", name=f"O{b}_{h}")
                    for jj in range(QS):
                        q = j0 + jj
                        # dst: O[p, jj, c*11+i] viewed [P, C, CTXW]
                        dst = O[:, jj, :].rearrange("p (c i) -> p c i", i=CTXW)
                        # src: X[p, q+i, c] viewed [P, C(stride 1), CTXW(stride C)]
                        src = X[b][:, q : q + CTXW, :].transpose([0, 2, 1])
                        mrow = M[b][:, q : q + 1]
                        e = eng_idx % 3
                        eng_idx += 1
                        if variant == "copyonly":
                            if e == 0:
                                nc.vector.tensor_copy(out=dst, in_=src)
                            elif e == 1:
                                nc.scalar.copy(out=dst, in_=src)
                            else:
                                nc.gpsimd.tensor_copy(out=dst, in_=src)
                        else:
                            if e == 0:
                                nc.vector.tensor_scalar_mul(
                                    out=dst, in0=src, scalar1=mrow
                                )
                            elif e == 1:
                                nc.scalar.mul(out=dst, in_=src, mul=mrow)
                            else:
                                nc.gpsimd.tensor_scalar_mul(
                                    out=dst, in0=src, scalar1=mrow
                                )
                    nc.sync.dma_start(
                        out=out_b[:, j0 : j0 + QS], in_=O[:, :, :]
                    )
    nc.compile()
    return nc


_NC_CACHE = {}


def _get_nc(variant: str = "full"):
    if variant not in _NC_CACHE:
        _NC_CACHE[variant] = _build_bass(variant)
    return _NC_CACHE[variant]


def _make_in_maps(x, lengths):
    x = np.asarray(x, dtype=np.float32)
    x_pad = np.zeros((B, TP, C), dtype=np.float32)
    x_pad[:, LEFT : LEFT + T, :] = x
    lengths = np.asarray(lengths, dtype=np.float32)
    len_abs = np.round(np.float32(T) * lengths).astype(np.int32)
    mask = (np.arange(T, dtype=np.int32)[None, :] < len_abs[:, None]).astype(
        np.float32
    )  # [B, T]
    return [
        {
            "xp": x_pad[c * B_LOC : (c + 1) * B_LOC],
            "mask": np.ascontiguousarray(mask[c * B_LOC : (c + 1) * B_LOC]),
        }
        for c in range(N_CORES)
    ]


def _run(x, lengths, variant: str = "full", **spmd_kwargs):
    res = run_bass_kernel_spmd(
        _get_nc(variant),
        _make_in_maps(x, lengths),
        list(range(N_CORES)),
        **spmd_kwargs,
    )
    out = np.concatenate([r["out"] for r in res.results], axis=0)
    return out, res


def kernel(x, lengths):
    out, _ = _run(x, lengths)
    return out


# revision 4
# speedup vs baseline: 1.2461x; 1.2461x over previous
"""Sliding context-window gather kernel for Trainium2 (Bass/Tile).

Computes, for x[B=32, T=2000, C=80] and lengths[B]:
    out[b, t, c*11 + i] = x[b, t + i - 5, c]          (zero outside [0, T))
                          * (t < round(T * lengths[b]))
i.e. an 11-tap sliding-window gather along T with channel-major
interleave, masked by per-sample length.

Sharding: pure data-parallel, 4 samples per core across 8 cores.

Layout: host zero-pads x by 5 rows on each side of T; each sample is
loaded into SBUF as an overlapping-window view [125p x 26r x 80c] (one
DMA; partition p holds padded rows 16p..16p+25 => t = 16p-5..16p+20).

Compute: one fused op per folded q-row builds the full 880-wide
interleaved+masked output row contiguously:
    O[p, (c,i)] = X[p, q+i, c] * mask[p, q]
via tensor_scalar multiply with a per-partition scalar (the mask value)
and a transposed source access pattern (c outer stride 1, i inner
stride 80).  Rows are split between the vector and scalar(ACT) engines
(~1 us/row each, measured); aggregate compute is ~4x faster than the
store stream, so it fully hides.

DMA: this environment's HWDGE queues (sync/scalar) share only 5 SDMA
engines (~120 GB/s combined), while the gpsimd SWDGE queue spreads
across all 16 engines.  All input loads and output stores therefore go
through nc.gpsimd (SWDGE); loads are prefetched up front, stores stream
out per 4-row chunk (1.76 MB contiguous-per-partition transfers).  The
Pool engine runs no compute so its Q7 cores are free for descriptor
generation.
"""

import numpy as np

import concourse.mybir as mybir
from concourse import bacc, bass
from concourse.ap import AP
from concourse.bass_utils import run_bass_kernel_spmd
from concourse.tile import TileContext

LEFT = 5
RIGHT = 5
CTXW = LEFT + RIGHT + 1  # 11
B, T, C = 32, 2000, 80
W = C * CTXW  # 880
N_CORES = 8
B_LOC = B // N_CORES  # 4 samples per core
P = 125  # SBUF partitions used per sample fold
Q = 16   # consecutive t rows per partition (P * Q == T)
QG = Q + LEFT + RIGHT  # 26 rows per partition incl. halo
TP = T + LEFT + RIGHT  # padded time length
F32 = mybir.dt.float32

assert P * Q == T

QS = 4        # q-rows per store chunk
NOBUF = 8     # store-chunk ring depth
N_ACT = 5     # q-rows per sample computed on the scalar(ACT) engine


def _build_bass(variant: str = "full"):
    nc = bacc.Bacc()
    xp_dram = nc.declare_dram_parameter("xp", [B_LOC, TP, C], F32, isOutput=False)
    msk = nc.declare_dram_parameter("mask", [B_LOC, T], F32, isOutput=False)
    out = nc.declare_dram_parameter("out", [B_LOC, T, W], F32, isOutput=True)

    store_eng = {"full": "gpsimd", "syncstore": "sync"}.get(variant, "gpsimd")

    with TileContext(nc) as tc:
        with (
            tc.tile_pool(name="xpool", bufs=1) as xpool,
            tc.tile_pool(name="mpool", bufs=1) as mpool,
            tc.tile_pool(name="opool", bufs=1) as opool,
        ):
            X = [None] * B_LOC
            M = [None] * B_LOC
            # prefetch all samples' inputs up front on the SWDGE queue so
            # they are ahead of every store in queue order
            for b in range(B_LOC):
                X[b] = xpool.tile([P, QG, C], F32, tag=f"X{b}", name=f"X{b}")
                M[b] = mpool.tile([P, Q], F32, tag=f"M{b}", name=f"M{b}")
                window = AP(
                    xp_dram[b].tensor,
                    b * TP * C,
                    [[Q * C, P], [C, QG], [1, C]],
                )
                nc.gpsimd.dma_start(out=X[b], in_=window)
                nc.gpsimd.dma_start(
                    out=M[b], in_=msk[b].rearrange("(p q) -> p q", q=Q)
                )

            ochunk = 0
            for b in range(B_LOC):
                out_b = out[b].rearrange("(p q) w -> p q w", q=Q)
                for h in range(Q // QS):
                    j0 = h * QS
                    O = opool.tile(
                        [P, QS, W], F32, tag=f"O{ochunk % NOBUF}", name=f"O{b}_{h}"
                    )
                    ochunk += 1
                    for jj in range(QS):
                        q = j0 + jj
                        # dst: O[p, jj, c*11+i] viewed [P, C, CTXW] (contig 880)
                        dst = O[:, jj, :].rearrange("p (c i) -> p c i", i=CTXW)
                        # src: X[p, q+i, c] viewed [P, C(s1), CTXW(s80)]
                        src = X[b][:, q : q + CTXW, :].transpose([0, 2, 1])
                        mrow = M[b][:, q : q + 1]
                        # interleave engines inside each chunk so both
                        # contribute to every chunk's critical path
                        if q % Q < Q - N_ACT:
                            nc.vector.tensor_scalar_mul(
                                out=dst, in0=src, scalar1=mrow
                            )
                        else:
                            nc.scalar.mul(out=dst, in_=src, mul=mrow)
                    dma = getattr(nc, store_eng)
                    dma.dma_start(out=out_b[:, j0 : j0 + QS], in_=O[:, :, :])
    nc.compile()
    return nc


_NC_CACHE = {}


def _get_nc(variant: str = "full"):
    if variant not in _NC_CACHE:
        _NC_CACHE[variant] = _build_bass(variant)
    return _NC_CACHE[variant]


def _make_in_maps(x, lengths):
    x = np.asarray(x, dtype=np.float32)
    x_pad = np.zeros((B, TP, C), dtype=np.float32)
    x_pad[:, LEFT : LEFT + T, :] = x
    lengths = np.asarray(lengths, dtype=np.float32)
    len_abs = np.round(np.float32(T) * lengths).astype(np.int32)
    mask = (np.arange(T, dtype=np.int32)[None, :] < len_abs[:, None]).astype(
        np.float32
    )  # [B, T]
    return [
        {
            "xp": x_pad[c * B_LOC : (c + 1) * B_LOC],
            "mask": np.ascontiguousarray(mask[c * B_LOC : (c + 1) * B_LOC]),
        }
        for c in range(N_CORES)
    ]


def _run(x, lengths, variant: str = "full", **spmd_kwargs):
    res = run_bass_kernel_spmd(
        _get_nc(variant),
        _make_in_maps(x, lengths),
        list(range(N_CORES)),
        **spmd_kwargs,
    )
    out = np.concatenate([r["out"] for r in res.results], axis=0)
    return out, res


def kernel(x, lengths):
    out, _ = _run(x, lengths)
    return out
